# revision 75
# baseline (speedup 1.0000x reference)
"""GCN classifier (3-layer GCNConv + residual + leaky_relu + global mean pool)
as a Bass/Tile kernel on 8 Trainium2 NeuronCores.

Sharding: nodes are range-partitioned across the 8 cores (6250 each, padded
to 6656); each core owns all edges whose destination lands in its range
(self-loops are materialized as explicit edges, which makes the GCN self-loop
term fall out of the same aggregation). Per layer, each core:
  - dma_gathers the fp16 feature rows y[src] (y = x * deg^-1/2, the halo
    exchange tensor) from a DRAM replica (layer 0: host-precomputed replicated
    input; layers 1-2: filled by an fp16 AllGather),
  - segment-sums them into its own nodes with PE indicator matmuls
    (indicator[e, n] = (dst_rel[e] == n) built on DVE via broadcast compare),
  - applies dst-side deg^-1/2 (host-fed broadcast tile), the shared 64x64
    weight (fp16), bias (fused into the Lrelu activation as a per-partition
    bias AP), residual (DVE add into PSUM) and leaky_relu, then transposes
    and AllGathers the rescaled fp16 result for the next layer.
deg^-1/2 and y0 = x * deg^-1/2 are host-side prep (the same numpy pass that
partitions edges); all three GCN layers and pooling run on device. The final
global-mean-pool partials (feature sums + counts per graph) are computed with
one more indicator matmul; the host sums the 8 partials and divides.
"""

import numpy as np

N = 50000
D = 64
G = 64
L = 3
C = 8
NPC = N // C            # 6250 real nodes per core
TIL = 64                # indicator width / node tile
GRP = 512               # nodes per PSUM group
NPC_PAD = 6656          # 13 * 512 = 52 * 128
NT = NPC_PAD // TIL     # 104 tiles
NGRP = NPC_PAD // GRP   # 13
TPG = GRP // TIL        # 8 tiles per group
HALF = C // 2 * NPC_PAD  # 26624 — first 4 cores' rows
PAD_DST = -1000.0
LRELU_DECOMP = False  # sim-only: bass_interp lacks Lrelu; decompose via Relu
TRACE = False         # test-only: capture NTFF profile, report exec_time_ns
LAST_RESULT = None    # test-only: BassKernelResults of the last run
SKIP_GATHER = False   # perf-probe: replace dma_gather with memset
SKIP_IND = False      # perf-probe: indicators via memset instead of is_equal
SKIP_AGG = False      # perf-probe: skip aggregation matmuls
NLAYERS = L           # perf-probe: layer count override
GATHER_SPLIT = 1      # sub-gathers per (group, half) batch
STOP_AFTER = ""       # perf-probe: truncate program after phase
                      # ("setup", "L0", "L1", "L2")
RDMA = False          # inter-layer halo exchange via remote_dma slot sends
                      # (XOR-permuted shard layout); False = ncfw AllGather.
                      # The remote_dma path is ~300us faster in the cost
                      # model and passes the 8-core simulator bit-for-bit,
                      # but this axon runtime's exec units go unrecoverable
                      # on the REMOTE_DMA_* ucode, so ship the collective.


def _host_prep(x, edge_index, batch):
    src = np.asarray(edge_index[0], dtype=np.int64)
    dst = np.asarray(edge_index[1], dtype=np.int64)
    # self loops as explicit edges
    loops = np.arange(N, dtype=np.int64)
    src = np.concatenate([src, loops])
    dst = np.concatenate([dst, loops])

    # padded row id in the halo buffer. With RDMA the shard layout on core c
    # is XOR-permuted: slot k holds core (c ^ k)'s shard (remote_dma slot-k
    # sends land on peer c^k; receiver r's slot k then holds r^k's shard).
    core = dst // NPC
    src_core = src // NPC
    slot = (src_core ^ core) if RDMA else src_core
    half = (slot >= C // 2).astype(np.int64)
    lrow = (slot % (C // 2)) * NPC_PAD + (src % NPC)  # row within its half
    dloc = dst % NPC
    tile = dloc // TIL
    drel = dloc % TIL

    order = np.lexsort((half, tile, core))
    core_s, tile_s, half_s = core[order], tile[order], half[order]
    lrow_s, drel_s = lrow[order], drel[order]

    key = (core_s * NT + tile_s) * 2 + half_s
    cnt = np.bincount(key, minlength=C * NT * 2).reshape(C, NT, 2)
    chunks = -(-cnt // 128)  # ceil div per (core, tile, half)
    plan = chunks.max(axis=0)          # [NT, 2] — shared across cores
    plan[:, 0] = np.maximum(plan[:, 0], 1)

    starts = np.zeros(C * NT * 2 + 1, np.int64)
    np.cumsum(cnt.reshape(-1), out=starts[1:])

    tot_chunks = int(plan.sum())
    tot_idx = tot_chunks * 128
    gidx = np.zeros((C, tot_idx), np.int16)
    dstrel = np.full((C, tot_chunks * 128), PAD_DST, np.float32)

    batch_chunks = np.zeros((NGRP, 2), np.int64)
    for g in range(NGRP):
        for h in range(2):
            batch_chunks[g, h] = plan[g * TPG:(g + 1) * TPG, h].sum()

    # fill per-core data in batch layout: for g, for h, for t in tiles(g)
    ci = 0
    for g in range(NGRP):
        for h in range(2):
            for tt in range(TPG):
                t = g * TPG + tt
                nch = int(plan[t, h])
                for c in range(C):
                    s = starts[(c * NT + t) * 2 + h]
                    e = starts[(c * NT + t) * 2 + h + 1]
                    n = e - s
                    gidx[c, ci * 128: ci * 128 + n] = lrow_s[s:e]
                    dstrel[c, ci * 128: ci * 128 + n] = drel_s[s:e]
                ci += nch
    assert ci == tot_chunks

    # wrap gather indices per batch block: logical i -> [i % 16, i // 16]
    gidx_w = np.zeros((C, 128, tot_idx // 16), np.int16)
    col = 0
    for g in range(NGRP):
        for h in range(2):
            nb = int(batch_chunks[g, h]) * 128
            blk = gidx[:, col * 16:col * 16 + nb].reshape(C, nb // 16, 16)
            gidx_w[:, :16, col:col + nb // 16] = np.transpose(blk, (0, 2, 1))
            col += nb // 16
    gidx_w = np.tile(gidx_w[:, :16, :], (1, 8, 1))

    dstrel_w = np.ascontiguousarray(
        dstrel.reshape(C, tot_chunks, 128).transpose(0, 2, 1)).astype(np.float16)

    # host-side normalization: deg (incl. self loop) -> dinv = deg^-1/2,
    # y0 = x * dinv in the padded AllGather layout (replicated to all cores)
    dl = np.bincount(dst, minlength=N).astype(np.float64)
    dinv_full = 1.0 / np.sqrt(np.maximum(dl, 1.0))
    y0 = (np.asarray(x, np.float64) * dinv_full[:, None]).astype(np.float32)
    y0_fulls, dinv_bcs, bvs = [], [], []
    b = np.asarray(batch, dtype=np.int64)
    for c in range(C):
        y0c = np.zeros((C * NPC_PAD, D), np.float32)
        for k in range(C):
            s = (c ^ k) if RDMA else k
            y0c[k * NPC_PAD: k * NPC_PAD + NPC] = y0[s * NPC:(s + 1) * NPC]
        y0_fulls.append(y0c)
        dp = np.ones(NPC_PAD, np.float16)
        dp[:NPC] = dinv_full[c * NPC:(c + 1) * NPC]
        dinv_bcs.append(np.ascontiguousarray(
            np.broadcast_to(dp[None, :], (128, NPC_PAD))))
        bv = np.full(NPC_PAD, PAD_DST, np.float16)
        bv[:NPC] = b[c * NPC:(c + 1) * NPC].astype(np.float16)
        bvs.append(bv.reshape(NPC_PAD // 128, 128).T.copy())  # [128, 52]
    return y0_fulls, dinv_bcs, bvs, gidx_w, dstrel_w, batch_chunks, plan, tot_chunks


_BUILD_CACHE = {}


def _build(batch_chunks, plan, tot_chunks):
    import concourse.bacc as bacc
    import concourse.tile as tile
    import concourse.mybir as mybir

    f32 = mybir.dt.float32
    f16 = mybir.dt.float16
    TOTC = tot_chunks
    MAXCH = int(batch_chunks.max())
    AF = mybir.ActivationFunctionType
    ALU = mybir.AluOpType

    nc = bacc.Bacc("TRN2", target_bir_lowering=False, debug=False, num_devices=C)

    if RDMA:
        arr_sem = nc.alloc_semaphore("halo_arr")   # remote: data arrived
        snd_sem = nc.alloc_semaphore("halo_snd")   # local: send buffer free
        ack_sem = nc.alloc_semaphore("halo_ack")   # remote: landing consumed
    # (inst, sem, val): raw HW-sem waits fused in AFTER Tile scheduling.
    # The schedule-time simulator cannot model remote_dma semaphore updates
    # (no_exec cost model gap) and would deadlock on them; at runtime the
    # updates come from the peers' SDMA engines.
    rdma_waits = []

    _ORDER = ["setup", "L0", "L1", "L2", "pool"]

    def _runs(stage):
        if not STOP_AFTER:
            return True
        return _ORDER.index(stage) <= _ORDER.index(STOP_AFTER)

    iota_c = nc.inline_tensor(
        np.tile(np.arange(TIL, dtype=np.float16)[None, :], (128, 1)), name="iota_c")
    iota32_c = nc.inline_tensor(
        np.tile(np.arange(TIL, dtype=np.float32)[None, :], (128, 1)), name="iota32_c")
    id_c = nc.inline_tensor(np.eye(128, dtype=np.float16), name="id_c")

    # chunk/idx col base per (g, h) batch
    cbase = np.zeros((NGRP, 2), np.int64)
    acc = 0
    for g in range(NGRP):
        for h in range(2):
            cbase[g, h] = acc
            acc += int(batch_chunks[g, h])
    # chunk offset of tile tt within batch (g, h)
    toff = np.zeros((NGRP, 2, TPG), np.int64)
    for g in range(NGRP):
        for h in range(2):
            o = 0
            for tt in range(TPG):
                toff[g, h, tt] = o
                o += int(plan[g * TPG + tt, h])

    with tile.TileContext(nc) as tc:
        with tc.tile_pool(name="dram", bufs=1, space="DRAM") as dram, \
             tc.tile_pool(name="per", bufs=1) as per, \
             tc.tile_pool(name="wrk", bufs=2 if RDMA else 4) as wrk, \
             tc.tile_pool(name="sml", bufs=2) as sml, \
             tc.tile_pool(name="ps", bufs=2, space="PSUM") as ps:

            y0_t = dram.tile([C * NPC_PAD, D], f32, kind="ExternalInput", name="y0_full", uniquify=False)
            gidx_t = dram.tile([128, TOTC * 8], mybir.dt.int16, kind="ExternalInput", name="gidx", uniquify=False)
            dstrel_t = dram.tile([128, TOTC], f16, kind="ExternalInput", name="dstrel", uniquify=False)
            dinvbc_t = dram.tile([128, NPC_PAD], f16, kind="ExternalInput", name="dinv_bc", uniquify=False)
            batchv_t = dram.tile([128, NPC_PAD // 128], f16, kind="ExternalInput", name="batchv", uniquify=False)
            Ws_t = dram.tile([L, D, D], f16, kind="ExternalInput", name="Ws", uniquify=False)
            bs_t = dram.tile([L, D], f32, kind="ExternalInput", name="bs", uniquify=False)
            out_t = dram.tile([D + 1, G], f32, kind="ExternalOutput", name="out_partial", uniquify=False)

            if RDMA:
                y_shard = None
                y_full = [y0_t] + [dram.tile([C * NPC_PAD, D], f32, kind="Internal",
                                             name=f"y_full{l}")
                                   for l in (1, 2)]
            else:
                NTRIM = 6272  # 49*128 >= NPC: all real rows, less halo bytes
                SA, SB = 4864, 1408  # uneven split: big half's expansion hides
                                     # under the small collective; small
                                     # tail minimizes post-AG serial work
                y_shard = [None] + [dram.tile([NPC_PAD, D], f16, kind="Internal",
                                              name=f"y_shard{l}")
                                    for l in (1, 2)]
                y_f16a = [None] + [dram.tile([C * SA, D], f16, kind="Internal",
                                             addr_space="Shared", name=f"y_f16a_{l}")
                                   for l in (1, 2)]
                y_f16b = [None] + [dram.tile([C * SB, D], f16, kind="Internal",
                                             addr_space="Shared", name=f"y_f16b_{l}")
                                   for l in (1, 2)]
                y_full = [y0_t] + [dram.tile([C * NPC_PAD, D], f32, kind="Internal",
                                             name=f"y_full{l}")
                                   for l in (1, 2)]

            # ---- persistent SBUF ----
            iota_sb = per.tile([128, TIL], f16)
            nc.sync.dma_start(iota_sb[:], iota_c[:])
            iota32_sb = per.tile([128, TIL], f32)
            nc.sync.dma_start(iota32_sb[:], iota32_c[:])
            id_sb = per.tile([128, 128], f16)
            nc.sync.dma_start(id_sb[:], id_c[:])
            dstrel_sb = per.tile([128, TOTC], f16)
            nc.sync.dma_start(dstrel_sb[:], dstrel_t[:])
            dinv_sb = per.tile([128, NPC_PAD], f16)
            nc.sync.dma_start(dinv_sb[:], dinvbc_t[:])
            batchv_sb = per.tile([128, NPC_PAD // 128], f16)
            nc.sync.dma_start(batchv_sb[:], batchv_t[:])
            Ws_sb = per.tile([2 * D, L, D], f16)
            nc.sync.dma_start(Ws_sb[0:D], Ws_t[:].rearrange("l k m -> k l m"))
            nc.sync.dma_start(Ws_sb[D:2 * D], Ws_t[:].rearrange("l k m -> k l m"))
            bs_sb = per.tile([D, L], f32)
            nc.sync.dma_start(bs_sb[:], bs_t[:].rearrange("l m -> m l"))

            NCG = NPC_PAD // 128  # 52
            y_nm = per.tile([128, NCG, D], f16)  # node-major staging
            if not RDMA:
                # zero the never-gathered pad tails of the f32 halo replicas
                # once (expansion only writes rows 0:NTRIM of each slot)
                ztail = per.tile([128, (NPC_PAD - NTRIM) // 128 * D], f32)
                nc.vector.memset(ztail[:], 0.0)
                for zl in (1, 2):
                    for j in range(C):
                        nc.sync.dma_start(
                            y_full[zl][j * NPC_PAD + NTRIM:(j + 1) * NPC_PAD, :]
                            .rearrange("(p g) f -> p (g f)", p=128),
                            ztail[:])
            if RDMA:
                landing = per.tile([128, C, NCG * D], f16)  # halo slots, f16
                ackbuf = per.tile([128, 2], f16)   # dummy payload for acks
                ackrecv = per.tile([128, 2], f16)  # peers' ack payloads land here
            x3_aug = per.tile([128, NPC_PAD // 128, D + 1], f16)
            nc.vector.memset(x3_aug[:, :, D:D + 1], 1.0)
            xT = per.tile([D, NPC_PAD], f16)          # current x, feature-major
            zero_sb = per.tile([128, D], f16)
            nc.vector.memset(zero_sb[:], 0.0)

            def build_ind(g, h):
                nbc = int(batch_chunks[g, h])
                cb = int(cbase[g, h])
                ind = wrk.tile([128, MAXCH, TIL], f16, tag="ind")
                if SKIP_IND:
                    nc.vector.memset(ind[:, 0:nbc, :], 0.0)
                    return ind
                nc.vector.tensor_tensor(
                    out=ind[:, 0:nbc, :],
                    in0=iota_sb[:, None, :].to_broadcast([128, nbc, TIL]),
                    in1=dstrel_sb[:, cb:cb + nbc, None].to_broadcast([128, nbc, TIL]),
                    op=ALU.is_equal)
                return ind

            # ================= layers =================
            _nl = NLAYERS
            if STOP_AFTER == "setup":
                _nl = 0
            elif STOP_AFTER == "L0":
                _nl = 1
            elif STOP_AFTER == "L1":
                _nl = 2
            pend_inds = None
            ynm_cps = []   # layer-1 y_nm writes (overwrite exchange-1's sends)
            n_exchanges = 0
            n_acks = 0
            for l in range(_nl):
                for g in range(NGRP):
                    agg_ps = ps.tile([128, 512], f32, space="PSUM", tag="agg")
                    msgs = []
                    for h in range(2):
                        nbc = int(batch_chunks[g, h])
                        cb = int(cbase[g, h])
                        nb = nbc * 128
                        gi = wrk.tile([128, MAXCH * 8], mybir.dt.int16, tag="gi")
                        nc.sync.dma_start(gi[:, 0:nb // 16],
                                          gidx_t[:, cb * 8:cb * 8 + nb // 16])
                        m = wrk.tile([128, MAXCH, D], f32, tag="msgs")
                        src_ap = y_full[l][HALF:, :] if h else y_full[l][0:HALF, :]
                        if SKIP_GATHER:
                            nc.vector.memset(m[:, 0:nbc, :], 0.125)
                        else:
                            splits = np.linspace(0, nbc, GATHER_SPLIT + 1).astype(int)
                            for s0, s1 in zip(splits[:-1], splits[1:]):
                                if s1 > s0:
                                    nsub = int(s1 - s0) * 128
                                    nc.gpsimd.dma_gather(
                                        m[:, s0:s1, :], src_ap,
                                        gi[:, s0 * 8:s0 * 8 + nsub // 16],
                                        nsub, nsub, D, single_packet=False)
                        mh = wrk.tile([128, MAXCH, D], f16, tag="msgsh")
                        nc.scalar.copy(out=mh[:, 0:nbc, :], in_=m[:, 0:nbc, :])
                        msgs.append(mh)
                    if g == 0 and pend_inds is not None:
                        inds = pend_inds
                        pend_inds = None
                    else:
                        inds = [build_ind(g, 0), build_ind(g, 1)]
                    if SKIP_AGG:
                        nc.tensor.matmul(out=agg_ps[0:D, :], lhsT=msgs[0][:, 0, :],
                                         rhs=inds[0][:, 0:8, :].rearrange("p c d -> p (c d)"),
                                         start=True, stop=True)
                        nc.tensor.matmul(out=agg_ps[D:128, :], lhsT=zero_sb[:],
                                         rhs=inds[0][:, 0:8, :].rearrange("p c d -> p (c d)"),
                                         start=True, stop=True, tile_position=(0, D))
                    else:
                        for tt in range(TPG):
                            t = g * TPG + tt
                            sl_t = slice(tt * TIL, (tt + 1) * TIL)
                            clist = [(h, j) for h in (0, 1)
                                     for j in range(int(plan[t, h]))]
                            npar = [(len(clist) + 1) // 2, len(clist) // 2]
                            cnt_p = [0, 0]
                            for ic, (h, j) in enumerate(clist):
                                p = ic % 2
                                jj = int(toff[g, h, tt]) + j
                                nc.tensor.matmul(
                                    out=agg_ps[D * p:D * p + D, sl_t],
                                    lhsT=msgs[h][:, jj, :], rhs=inds[h][:, jj, :],
                                    start=(cnt_p[p] == 0), stop=(cnt_p[p] == npar[p] - 1),
                                    tile_position=(0, D) if p else None,
                                    skip_group_check=True)
                                cnt_p[p] += 1
                            if npar[1] == 0:
                                nc.tensor.matmul(
                                    out=agg_ps[D:2 * D, sl_t], lhsT=zero_sb[:],
                                    rhs=inds[0][:, int(toff[g, 0, tt]), :],
                                    start=True, stop=True, tile_position=(0, D),
                                    skip_group_check=True)
                    # epilogue for this 512-node group
                    sl = slice(g * 512, (g + 1) * 512)
                    rhs_sb = sml.tile([128, 512], f16, tag="rhs")
                    nc.vector.tensor_tensor(out=rhs_sb[0:D, :], in0=agg_ps[0:D, :],
                                            in1=dinv_sb[0:D, sl], op=ALU.mult)
                    nc.vector.tensor_tensor(out=rhs_sb[D:2 * D, :],
                                            in0=agg_ps[D:2 * D, :],
                                            in1=dinv_sb[D:2 * D, sl], op=ALU.mult)
                    tr_ps = ps.tile([D, 512], f32, space="PSUM", tag="tr")
                    nc.tensor.matmul(out=tr_ps[:], lhsT=Ws_sb[:, l, :],
                                     rhs=rhs_sb[:], start=True, stop=True)
                    if l > 0:
                        nc.vector.tensor_tensor(out=tr_ps[:], in0=tr_ps[:],
                                                in1=xT[:, sl], op=ALU.add)
                    if LRELU_DECOMP:
                        bb = sml.tile([D, 512], f32, tag="lr0", bufs=1)
                        nc.vector.tensor_tensor(
                            out=bb[:], in0=tr_ps[:],
                            in1=bs_sb[:, l:l + 1].to_broadcast([D, 512]),
                            op=ALU.add)
                        r_sb = sml.tile([D, 512], f32, tag="lr1", bufs=1)
                        nc.scalar.activation(out=r_sb[:], in_=bb[:], func=AF.Relu)
                        t_sb = sml.tile([D, 512], f32, tag="lr2", bufs=1)
                        nc.scalar.activation(out=t_sb[:], in_=bb[:],
                                             func=AF.Copy, scale=0.01)
                        nc.vector.scalar_tensor_tensor(
                            out=xT[:, sl], in0=r_sb[:], scalar=0.99, in1=t_sb[:],
                            op0=ALU.mult, op1=ALU.add)
                    else:
                        nc.scalar.activation(out=xT[:, sl], in_=tr_ps[:],
                                             func=AF.Lrelu, alpha=0.01,
                                             bias=bs_sb[:, l:l + 1])
                    tp_ps = ps.tile([128, 256], f16, space="PSUM", tag="tp")
                    if l < L - 1:
                        yT = sml.tile([D, 512], f16, tag="yT")
                        nc.vector.tensor_tensor(out=yT[:], in0=xT[:, sl],
                                                in1=dinv_sb[0:D, sl], op=ALU.mult)
                        for k in range(4):
                            nc.tensor.transpose(out=tp_ps[:, k * D:(k + 1) * D],
                                                in_=yT[:, k * 128:(k + 1) * 128],
                                                identity=id_sb[0:D, 0:D])
                        cp = nc.scalar.copy(
                            out=y_nm[:, g * 4:(g + 1) * 4, :],
                            in_=tp_ps[:].rearrange("p (g f) -> p g f", f=D))
                        if RDMA and l == 1:
                            ynm_cps.append(cp)
                    else:
                        for k in range(4):
                            nc.tensor.transpose(out=tp_ps[:, k * D:(k + 1) * D],
                                                in_=xT[:, g * 512 + k * 128: g * 512 + (k + 1) * 128],
                                                identity=id_sb[0:D, 0:D])
                        nc.scalar.copy(
                            out=x3_aug[:, g * 4:(g + 1) * 4, 0:D],
                            in_=tp_ps[:].rearrange("p (g f) -> p g f", f=D))
                if l < L - 1 and not RDMA:
                    nc.sync.dma_start(
                        y_shard[l + 1][:].rearrange("(g p) f -> p g f", p=128), y_nm[:])
                    pend_inds = [build_ind(0, 0), build_ind(0, 1)]
                    # two row-split AllGathers: the first half's f16->f32
                    # expansion overlaps the second collective's transfer
                    nc.gpsimd.collective_compute(
                        "AllGather", ALU.bypass, replica_groups=[list(range(C))],
                        ins=[y_shard[l + 1][0:SA, :]],
                        outs=[y_f16a[l + 1][:]])
                    nc.gpsimd.collective_compute(
                        "AllGather", ALU.bypass, replica_groups=[list(range(C))],
                        ins=[y_shard[l + 1][SA:NTRIM, :]],
                        outs=[y_f16b[l + 1][:]])
                    # expand each half to the f32 gather replica, one
                    # source-shard chunk at a time (DMA in / DVE cast / DMA
                    # out pipeline through SBUF). Rows NTRIM..NPC_PAD of each
                    # f32 slot are never gathered (pad gidx entries hit row 0).
                    for src, rows, roff in ((y_f16a, SA, 0), (y_f16b, SB, SA)):
                        nct = rows // 128
                        for j in range(C):
                            s16 = sml.tile([128, nct, D], f16, tag="x16")
                            nc.sync.dma_start(
                                s16[:].rearrange("p g f -> p (g f)"),
                                src[l + 1][j * rows:(j + 1) * rows, :]
                                .rearrange("(p g) f -> p (g f)", p=128))
                            s32 = sml.tile([128, nct, D], f32, tag="x32")
                            nc.vector.tensor_copy(out=s32[:], in_=s16[:])
                            nc.sync.dma_start(
                                y_full[l + 1][j * NPC_PAD + roff:
                                              j * NPC_PAD + roff + rows, :]
                                .rearrange("(p g) f -> p (g f)", p=128),
                                s32[:].rearrange("p g f -> p (g f)"))
                elif l < L - 1:
                    ex = l + 1
                    # slot-j send lands on peer (self ^ j) at landing[:, j, :]
                    # (Q7 XORs the relative (0, j) dest with its own tpb_idx),
                    # so receiver r's slot j holds r^j's shard — the same XOR
                    # permutation the host baked into gidx / y0_full.
                    preps = []
                    for j in range(C):
                        rd = [None] * C
                        rd[j] = (0, j)
                        preps.append(nc.gpsimd.remote_dma_broadcast(
                            out_ap=landing[:, j, :],
                            in_ap=y_nm[:].rearrange("p g f -> p (g f)"),
                            remote_sem=arr_sem, local_sem=snd_sem, rdests=rd))
                    tr = nc.gpsimd.trigger_dma(count=None)
                    if ex > 1:
                        # peers' landings free (acked last exchange)
                        rdma_waits.append(([tr], ack_sem, 16 * (ex - 1)))
                    pend_inds = [build_ind(0, 0), build_ind(0, 1)]
                    st = None
                    casts = []
                    for k in range(C):
                        st = sml.tile([128, NCG, D], f32, tag="stage")
                        casts.append(nc.vector.tensor_copy(
                            out=st[:].rearrange("p g f -> p (g f)"),
                            in_=landing[:, k, :]))
                        nc.sync.dma_start(
                            y_full[l + 1][k * NPC_PAD:(k + 1) * NPC_PAD, :]
                            .rearrange("(g p) f -> p g f", p=128),
                            st[:])
                    # all 8 shards arrived (2 per send, incl. self)
                    rdma_waits.append((casts, arr_sem, 16 * ex))
                    n_exchanges += 1
                    if ex < L - 1:
                        # landing consumed -> ack all peers (+2 each, incl.
                        # self). The ack carries a dummy payload read from the
                        # last stage tile so Tile chains it after the casts.
                        nc.vector.tensor_copy(out=ackbuf[:], in_=st[:, 0, 0:2])
                        nc.gpsimd.remote_dma_broadcast(
                            out_ap=ackrecv[:], in_ap=ackbuf[:],
                            remote_sem=ack_sem, local_sem=snd_sem,
                            rdests=[(0, j) for j in range(C)])
                        nc.gpsimd.trigger_dma(count=None)
                        n_acks += 1

            if RDMA and ynm_cps:
                # y_nm overwrite fence: exchange-1's sends (data 8x16 +
                # ack 16) finished reading y_nm before layer 1 rewrites it
                rdma_waits.append((ynm_cps, snd_sem, 144))

            # ================= pooling =================
            if _runs("pool"):
                NCG = NPC_PAD // 128  # 52
                pind = wrk.tile([128, NCG, G], f16, tag="pind")
                nc.vector.tensor_tensor(
                    out=pind[:],
                    in0=iota_sb[:, None, :].to_broadcast([128, NCG, G]),
                    in1=batchv_sb[:, :, None].to_broadcast([128, NCG, G]),
                    op=ALU.is_equal)
                pool_ps = ps.tile([D + 1, G], f32, space="PSUM", tag="tr")
                for t in range(NCG):
                    nc.tensor.matmul(out=pool_ps[:], lhsT=x3_aug[:, t, :], rhs=pind[:, t, :],
                                     start=(t == 0), stop=(t == NCG - 1))
                pool_sb = sml.tile([D + 1, G], f32, tag="dr")
                nc.vector.tensor_copy(out=pool_sb[:], in_=pool_ps[:])
                nc.sync.dma_start(out_t[:], pool_sb[:])

    # Insert the raw cross-core waits post-scheduling as standalone
    # EventSemaphore instructions placed directly before the earliest
    # consumer in its (already scheduled) engine stream.
    if RDMA and rdma_waits:
        import concourse.bass as _bass
        pos = {}
        for bbw in nc.bb_map.values():
            for i, inst in enumerate(bbw.bb.instructions):
                pos[inst.name] = (bbw.bb, i)
        inserts = []
        for targets, sem, val in rdma_waits:
            spots = [(pos[t.ins.name], t) for t in targets]
            (bbobj, idx), t = min(spots, key=lambda x: x[0][1])
            ev = mybir.InstEventSemaphore(
                name=nc.get_next_instruction_name(), ins=[], outs=[],
                engine=t.ins.engine)
            _bass._bass_rust.wait_op(ev, sem, val, "sem-ge", True)
            inserts.append((bbobj, idx, ev))
        # end-of-program: drain all halo traffic, then reset the sems so
        # back-to-back invocations of the cached program start from zero
        endbb = next(b.bb for n, b in nc.bb_map.items() if n.endswith("_end"))
        arr_t = 16 * n_exchanges
        snd_t = 128 * n_exchanges + 16 * n_acks
        ack_t = 16 * n_acks
        if n_exchanges:
            ev1 = mybir.InstEventSemaphore(
                name=nc.get_next_instruction_name(), ins=[], outs=[],
                engine=mybir.EngineType.Pool)
            _bass._bass_rust.wait_op(ev1, arr_sem, arr_t, "sem-ge", True)
            _bass._bass_rust.wait_op(ev1, snd_sem, snd_t, "sem-ge", True)
            if ack_t:
                ev2 = mybir.InstEventSemaphore(
                    name=nc.get_next_instruction_name(), ins=[], outs=[],
                    engine=mybir.EngineType.Pool)
                _bass._bass_rust.wait_op(ev2, ack_sem, ack_t, "sem-ge", True)
                inserts.append((endbb, 0, ev2))
            inserts.append((endbb, 0, ev1))
            # widen Tile's end-of-program semaphore reset to cover the halo
            # sems so back-to-back invocations start from zero
            lo = min(arr_sem.num, snd_sem.num, ack_sem.num)
            for inst in endbb.instructions:
                if getattr(inst, "is_reset_sema", None):
                    assert inst.reset_range_start >= lo
                    inst.reset_range_start = lo
                if "RANGE_CLEAR" in str(getattr(inst, "op_name", "")):
                    d = inst.ant_dict
                    assert d["range_first"] >= lo
                    d["range_first"] = lo
        # apply per-bb in descending index order so indices stay valid
        for bbobj, idx, ev in sorted(
                inserts, key=lambda x: x[1], reverse=True):
            lst = bbobj.instructions
            lst.insert(idx, ev)
            bbobj.instructions = lst
            nc.inst_map[ev.name] = ev

    nc.compile()
    return nc


def kernel(x, edge_index, batch, Ws, bs):
    from concourse.bass_utils import run_bass_kernel_spmd

    x = np.asarray(x, np.float32)
    Ws_np = np.asarray(Ws, np.float16)
    bs_np = np.asarray(bs, np.float32)

    (y0_fulls, dinv_bcs, bvs, gidx_w, dstrel_w, batch_chunks, plan,
     tot_chunks) = _host_prep(x, edge_index, batch)

    key = (batch_chunks.tobytes(), plan.tobytes())
    if key not in _BUILD_CACHE:
        _BUILD_CACHE[key] = _build(batch_chunks, plan, tot_chunks)
    nc = _BUILD_CACHE[key]

    in_maps = []
    for c in range(C):
        in_maps.append({
            "y0_full": y0_fulls[c],
            "gidx": np.ascontiguousarray(gidx_w[c]),
            "dstrel": np.ascontiguousarray(dstrel_w[c]),
            "dinv_bc": dinv_bcs[c],
            "batchv": np.ascontiguousarray(bvs[c]),
            "Ws": Ws_np,
            "bs": bs_np,
        })
    res = None
    for attempt in range(3):
        try:
            res = run_bass_kernel_spmd(nc, in_maps, core_ids=list(range(C)),
                                       trace=TRACE)
            break
        except Exception:
            if attempt == 2:
                raise
            import time
            time.sleep(5.0)
    global LAST_RESULT
    LAST_RESULT = res

    total = np.zeros((D + 1, G), np.float64)
    for c in range(C):
        total += res.results[c]["out_partial"].astype(np.float64)
    sums = total[:D]                    # [feat, graph]
    counts = np.maximum(total[D], 1.0)  # [graph]
    pooled = (sums / counts[None, :]).T.astype(np.float32)
    return pooled
